# revision 9
# baseline (speedup 1.0000x reference)
"""CondConv2d Trainium2 kernel (bf16 data path).

Data-parallel over batch: 32 samples -> 8 cores x 4 samples.
Per core:
  - x arrives host-prepadded in bf16 ([128, 2, XCOLS] per sample, zero
    margins), one contiguous DMA per sample; the expert table arrives
    bf16 and stays SBUF-resident (8 x [128, 4608]).
  - DMA order: consts, xpad[0], wtab[0..7], xpad[1..3] — minimizes the
    time until kgen[0] (which needs the full table) is buildable.
  - per-sample attention: pooled sums reduce over the padded bf16 image
    (co=0 on DVE, co=1 on Pool; pad zeros don't change the sum), tiny
    per-sample MLP + softmax + PE broadcast.  MLP matmuls for samples
    1..3 are interleaved between conv chunks so the in-order PE never
    stalls on late pooled sums.
  - per-sample kernels K_s = sum_w att[s,w] * W_w: column-split between
    DVE (tensor_scalar_mul 4x mode + tensor_tensor add 2x mode) and
    Pool (fused scalar_tensor_tensor), ~16us per sample, sample-major
    so conv for sample 0 starts as soon as kgen[0] is ready.
  - 3x3 conv as 18-matmul PSUM accumulation groups (2 cin chunks x 9
    taps) in bf16 (1 cycle/row vs 4 for fp32), bias added on ScalarE
    during the PSUM->SBUF copy, bf16 output DMA'd (issued from the
    Activation engine's DGE to keep SP free) as full padded rows;
    host strips pad cols and upcasts to fp32.
"""

import numpy as np

B, CIN, H, W = 32, 256, 56, 56
COUT, KH, KW = 256, 3, 3
ATT_IN, NW = 512, 8
NCORES = 8
BS = B // NCORES  # 4

WP = W + 2                 # 58: padded row width (lpad + 56 + rpad)
PCOLS = H * WP             # 3248
MARG = WP + 1              # 59: left/right margin so tap windows stay in-bounds
XCOLS = MARG + PCOLS + MARG  # 3366
RT = 8                     # output rows per PSUM tile
NT = H // RT               # 7
NFREE = RT * WP            # 464 (<= 512 fp32 PSUM bank)
KFREE = 2 * 9 * COUT       # 4608 free cols of a generated kernel
KHALF = KFREE // 2         # DVE/Pool column split of the kgen FMA

_nc_cache = {}


def _split_waits(nc, max_inline=1):
    """Hoist inline sem waits beyond max_inline into standalone NoOps.

    This walrus build rejects instructions carrying more than one inline
    sync wait ("Too many sync wait commands"); standalone single-wait
    NoOps on the same engine are semantically identical and compile.
    """
    import copy

    import concourse.mybir as mybir

    cnt = 0
    for f in nc.m.functions:
        new_blocks = []
        for blk in f.blocks:
            out = []
            for inst in blk.instructions:
                si = inst.sync_info
                waits = list(si.on_wait) if si is not None else []
                if len(waits) > max_inline:
                    keep = waits[-max_inline:]
                    hoist = waits[:-max_inline]
                    for w in hoist:
                        nop = mybir.InstNoOp(name=f"WSPL-{cnt}", ins=[], outs=[])
                        cnt += 1
                        nop.engine = inst.engine
                        nop.sync_info = mybir.SyncInfo(on_wait=[w], on_update=[])
                        out.append(nop)
                    inst.sync_info = mybir.SyncInfo(
                        on_wait=keep, on_update=list(si.on_update)
                    )
                out.append(inst)
            new_blocks.append(copy.replace(blk, instructions=out))
        try:
            f.blocks[:] = new_blocks
        except TypeError:
            for i, nb in enumerate(new_blocks):
                f.blocks[i] = nb
    return cnt


def _build_nc(reps=1, split_waits=True):
    from contextlib import ExitStack

    import concourse.bass as bass
    import concourse.mybir as mybir
    import concourse.tile as tile

    fp32 = mybir.dt.float32
    bf16 = mybir.dt.bfloat16
    AF = mybir.ActivationFunctionType
    ALU = mybir.AluOpType
    AX = mybir.AxisListType

    nc = bass.Bass()

    # cblob packs the [128, ...] fp32 constants: attxT(16) w1t(1024)
    # w2t(512) wat(16) wb(16) -> 1584 cols
    cblob_d = nc.declare_dram_parameter("cblob", [128, 1584], fp32, isOutput=False)
    b12_d = nc.declare_dram_parameter("b12", [1, 256], fp32, isOutput=False)
    batt_d = nc.declare_dram_parameter("batt", [1, NW], fp32, isOutput=False)
    xpad_d = nc.declare_dram_parameter("xpad", [BS, 128, 2 * XCOLS], bf16, isOutput=False)
    wrep_d = nc.declare_dram_parameter("wrep", [NW, 128, KFREE], bf16, isOutput=False)
    y_d = nc.declare_dram_parameter("y", [BS, COUT, NT, NFREE], bf16, isOutput=True)

    with ExitStack() as ctx:
        tc = ctx.enter_context(tile.TileContext(nc))
        constp = ctx.enter_context(tc.tile_pool(name="const", bufs=1))
        wtabp = ctx.enter_context(tc.tile_pool(name="wtab", bufs=1))
        kgenp = ctx.enter_context(tc.tile_pool(name="kgen", bufs=1))
        xpadp = ctx.enter_context(tc.tile_pool(name="xpad", bufs=1))
        tmpp = ctx.enter_context(tc.tile_pool(name="tmp", bufs=2))
        osbp = ctx.enter_context(tc.tile_pool(name="osb", bufs=4))
        psum_s = ctx.enter_context(tc.tile_pool(name="psum_s", bufs=2, space="PSUM"))
        psum_c = ctx.enter_context(tc.tile_pool(name="psum_c", bufs=6, space="PSUM"))

        # --- constants (loaded once, reused across reps)
        # consts ride the Activation engine's DGE so SP's queue starts on
        # xpad/wtab (the critical path) immediately
        cblob = constp.tile([128, 1584], fp32)
        nc.scalar.dma_start(cblob[:], cblob_d[:])
        b12 = constp.tile([1, 256], fp32)
        nc.scalar.dma_start(b12[:], b12_d[:])
        batt = constp.tile([1, NW], fp32)
        nc.scalar.dma_start(batt[:], batt_d[:])
        attxT = cblob[:, 0:16].rearrange("p (k s) -> p k s", k=4)
        w1t = cblob[:, 16:1040].rearrange("p (k m) -> p k m", k=4)
        w2t = cblob[:, 1040:1552].rearrange("p (k m) -> p k m", k=2)
        wat = cblob[:, 1552:1568].rearrange("p (k m) -> p k m", k=2)
        wb = cblob[:, 1568:1584].rearrange("p (k m) -> p k m", k=2)
        ones11 = constp.tile([1, 1], fp32)
        nc.vector.memset(ones11[:], 1.0)
        ones_bc = constp.tile([1, 128], fp32)
        nc.vector.memset(ones_bc[:], 1.0)

        # --- persistent state
        kgen = [kgenp.tile([128, KFREE], bf16, tag=f"kgen{s}", name=f"kgen{s}") for s in range(BS)]
        xpad = [
            xpadp.tile([128, 2, XCOLS], bf16, tag=f"xpad{s}", name=f"xpad{s}")
            for s in range(BS)
        ]
        wtab = [
            wtabp.tile([128, KFREE], bf16, tag=f"wtab{w}", name=f"wtab{w}")
            for w in range(NW)
        ]
        pooledT = constp.tile([128, 2, BS], fp32)
        mixT = constp.tile([128, 2, BS], fp32)
        attbc = constp.tile([128, BS * NW], fp32)
        biasall = constp.tile([128, 2, BS], fp32)
        actsink = constp.tile([128, XCOLS], bf16)
        p3s = constp.tile([1, NW], fp32)
        mx = constp.tile([1, 1], fp32)
        nmx = constp.tile([1, 1], fp32)
        ex = constp.tile([1, NW], fp32)
        sm = constp.tile([1, 1], fp32)
        rcp = constp.tile([1, 1], fp32)
        att1 = constp.tile([1, NW], fp32)

        def att_ap(s, w):
            return attbc[:, s * NW + w : s * NW + w + 1]

        def issue_dmas(r):
            # SP issue order = critical-path order: xpad[0], xpad[1] (first
            # two attention chains), the full expert table, then the rest
            for s in (0, 1):
                nc.sync.dma_start(
                    xpad[s][:], xpad_d[s].rearrange("p (c q) -> p c q", c=2)
                )
            for w in range(NW):
                nc.sync.dma_start(wtab[w][:], wrep_d[w])
            for s in (2, 3):
                nc.sync.dma_start(
                    xpad[s][:], xpad_d[s].rearrange("p (c q) -> p c q", c=2)
                )

        if True:
            def chain_pre(s):
                """Per-sample attention -> kgen[s] -> biasall[s]."""
                # pooled sums (pad zeros contribute nothing)
                nc.vector.reduce_sum(
                    pooledT[:, 0:1, s : s + 1], xpad[s][:, 0:1, :], axis=AX.X
                )
                nc.scalar.activation(
                    actsink[:], xpad[s][:, 1, :], AF.Copy,
                    accum_out=pooledT[:, 1:2, s : s + 1],
                )
                # two-layer MLP, batched-of-1 on the PE
                for c2 in range(2):
                    pm = psum_s.tile([128, 1], fp32, tag="mlps", name="pm")
                    nc.tensor.matmul(
                        pm[:], w1t[:, 0, c2 * 128 : (c2 + 1) * 128],
                        attxT[:, 0, s : s + 1], start=True, stop=False,
                    )
                    for ko in range(1, 4):
                        nc.tensor.matmul(
                            pm[:], w1t[:, ko, c2 * 128 : (c2 + 1) * 128],
                            attxT[:, ko, s : s + 1], start=False, stop=False,
                        )
                    for jo in range(2):
                        nc.tensor.matmul(
                            pm[:], w2t[:, jo, c2 * 128 : (c2 + 1) * 128],
                            pooledT[:, jo, s : s + 1], start=False, stop=False,
                        )
                    nc.tensor.matmul(
                        pm[:], b12[0:1, c2 * 128 : (c2 + 1) * 128], ones11[0:1, :],
                        start=False, stop=True,
                    )
                    nc.scalar.activation(mixT[:, c2, s : s + 1], pm[:], AF.Tanh)

                # worker logits on partition 0: [1, NW]
                p3 = psum_s.tile([1, NW], fp32, tag="mlps", name="p3")
                nc.tensor.matmul(
                    p3[:], mixT[:, 0, s : s + 1], wat[:, 0, :], start=True, stop=False
                )
                nc.tensor.matmul(
                    p3[:], mixT[:, 1, s : s + 1], wat[:, 1, :], start=False, stop=False
                )
                nc.tensor.matmul(
                    p3[:], ones11[0:1, :], batt[0:1, :], start=False, stop=True
                )
                nc.vector.tensor_copy(p3s[:], p3[:])

                # softmax over the 8 workers
                nc.vector.reduce_max(mx[:], p3s[:], axis=AX.X)
                nc.vector.tensor_scalar_mul(nmx[:], mx[:], -1.0)
                nc.scalar.activation(ex[:], p3s[:], AF.Exp, bias=nmx[:], scale=1.0)
                nc.vector.reduce_sum(sm[:], ex[:], axis=AX.X)
                nc.vector.reciprocal(rcp[:], sm[:])
                nc.vector.tensor_scalar_mul(att1[:], ex[:], rcp[:])

                # broadcast att row to all 128 partitions via PE
                pbc = psum_s.tile([128, NW], fp32, tag="mlps", name="pbc")
                nc.tensor.matmul(pbc[:], ones_bc[0:1, :], att1[0:1, :], start=True, stop=True)
                nc.vector.tensor_copy(attbc[:, s * NW : (s + 1) * NW], pbc[:])

                # kgen[s] on DVE: tensor_scalar_mul runs in 4x mode and
                # tensor_tensor add in 2x mode for packed bf16 — together
                # ~2.6x faster than the fused scalar_tensor_tensor chain
                # (and walrus rejects TensorScalarPtr on Pool)
                nc.vector.tensor_scalar_mul(
                    kgen[s][:], wtab[0][:], att_ap(s, 0)
                )
                for w in range(1, NW):
                    tmp = tmpp.tile([128, KFREE], bf16)
                    nc.vector.tensor_scalar_mul(
                        tmp[:], wtab[w][:], att_ap(s, w)
                    )
                    nc.vector.tensor_tensor(
                        kgen[s][:], kgen[s][:], tmp[:], ALU.add
                    )

                # conv bias: biasall[:, :, s] = sum_w att * wb
                nc.vector.tensor_scalar_mul(
                    biasall[:, :, s : s + 1], wb[:, :, 0:1], att_ap(s, 0)
                )
                for w in range(1, NW):
                    nc.vector.scalar_tensor_tensor(
                        biasall[:, :, s : s + 1], wb[:, :, w : w + 1], att_ap(s, w),
                        biasall[:, :, s : s + 1], ALU.mult, ALU.add,
                    )

            def conv_chunk(s, c):
                """One cout-half of sample s: 7 PSUM groups of 18 matmuls."""
                kgv = kgen[s][:].rearrange("p (co t m) -> p co t m", co=2, t=9)
                xb = xpad[s]
                bias_ap = biasall[:, c, s : s + 1]
                for rt in range(NT):
                    p0 = rt * RT * WP
                    ps = psum_c.tile([128, NFREE], fp32)
                    k = 0
                    for co in range(2):
                        for t in range(9):
                            dh, dw = t // 3 - 1, t % 3 - 1
                            qs = MARG + p0 + dh * WP + dw
                            nc.tensor.matmul(
                                ps[:],
                                kgv[:, co, t, c * 128 : (c + 1) * 128],
                                xb[:, co, qs : qs + NFREE],
                                start=(k == 0), stop=(k == 17),
                            )
                            k += 1
                    ob = osbp.tile([128, NFREE], bf16)
                    nc.scalar.activation(
                        ob[:], ps[:], AF.Identity, bias=bias_ap, scale=1.0
                    )
                    nc.scalar.dma_start(
                        y_d[s, c * 128 : (c + 1) * 128, rt, :], ob[:]
                    )

            # software-pipelined schedule over all (rep, sample) units: each
            # unit's attention/kgen chain is issued two conv-chunks ahead of
            # its conv — across rep boundaries too — so the in-order PE
            # never stalls on a late attention chain or table reload.
            seq = [(r, s) for r in range(reps) for s in range(BS)]
            N = len(seq)

            def chain_unit(k):
                r, s = seq[k]
                if s == 0:
                    issue_dmas(r)
                chain_pre(s)

            chain_unit(0)
            if N > 1:
                chain_unit(1)
            for k in range(N):
                conv_chunk(seq[k][1], 0)
                if k + 2 < N:
                    chain_unit(k + 2)
                conv_chunk(seq[k][1], 1)

    if split_waits:
        _split_waits(nc)
    return nc


def _prep_inputs(inputs):
    import ml_dtypes

    bf16 = ml_dtypes.bfloat16

    x = np.asarray(inputs["x"], np.float32)
    att_x = np.asarray(inputs["att_x"], np.float32)
    W_att1 = np.asarray(inputs["W_att1"], np.float32)
    b_att1 = np.asarray(inputs["b_att1"], np.float32)
    W_att2 = np.asarray(inputs["W_att2"], np.float32)
    b_att2 = np.asarray(inputs["b_att2"], np.float32)
    W_att = np.asarray(inputs["W_att"], np.float32)
    b_att = np.asarray(inputs["b_att"], np.float32)
    W_weight = np.asarray(inputs["W_weight"], np.float32)
    W_bias = np.asarray(inputs["W_bias"], np.float32)

    w1t = W_att1.T.reshape(4, 128, 256).transpose(1, 0, 2)  # [128, 4, 256]
    w2t = (W_att2 / float(H * W)).T.reshape(2, 128, 256).transpose(1, 0, 2)
    wat = W_att.T.reshape(2, 128, NW).transpose(1, 0, 2)
    wb = W_bias.reshape(2, 128, NW).transpose(1, 0, 2)
    b12 = np.ascontiguousarray((b_att1 + b_att2).reshape(1, 256))
    batt = np.ascontiguousarray(b_att.reshape(1, NW))
    # W_weight rows are [cout, cin, kh, kw] flattened; target [w, ci, (co2 t cout)]
    w5 = W_weight.reshape(COUT, 2, 128, 9, NW)
    wrep = np.ascontiguousarray(
        w5.transpose(4, 2, 1, 3, 0).reshape(NW, 128, KFREE)
    ).astype(bf16)

    # host-prepadded bf16 image: [B, 128 cin-part, 2 cin-chunk, XCOLS]
    xp = np.zeros((B, 128, 2, XCOLS), bf16)
    xv = xp[:, :, :, MARG + 1 : MARG + 1 + PCOLS].reshape(B, 128, 2, H, WP)[..., :W]
    xv[...] = x.reshape(B, 2, 128, H, W).transpose(0, 2, 1, 3, 4).astype(bf16)
    xp = xp.reshape(B, 128, 2 * XCOLS)

    in_maps = []
    for i in range(NCORES):
        axs = att_x[i * BS : (i + 1) * BS]  # [4, 512]
        attxT = axs.T.reshape(4, 128, BS).transpose(1, 0, 2)  # [128, 4, BS]
        cblob = np.concatenate(
            [
                attxT.reshape(128, 4 * BS)[:, : 4 * BS],
                w1t.reshape(128, 1024),
                w2t.reshape(128, 512),
                wat.reshape(128, 2 * NW),
                wb.reshape(128, 2 * NW),
            ],
            axis=1,
        ).astype(np.float32)
        in_maps.append(
            {
                "cblob": np.ascontiguousarray(cblob),
                "b12": b12,
                "batt": batt,
                "xpad": xp[i * BS : (i + 1) * BS],
                "wrep": wrep,
            }
        )
    return in_maps


def _unpack_y(y_bf16):
    """[B, COUT, NT, NFREE] bf16 padded rows -> [B, COUT, H, W] fp32."""
    y = np.asarray(y_bf16).reshape(B, COUT, NT, RT, WP)[..., 1 : 1 + W]
    return np.ascontiguousarray(y.reshape(B, COUT, H, W).astype(np.float32))


def kernel(**inputs):
    from concourse.bass_utils import run_bass_kernel_spmd

    if "nc" not in _nc_cache:
        _nc_cache["nc"] = _build_nc()
    in_maps = _prep_inputs(inputs)
    res = run_bass_kernel_spmd(_nc_cache["nc"], in_maps, core_ids=list(range(NCORES)))
    y = np.concatenate([np.asarray(res.results[i]["y"]) for i in range(NCORES)], axis=0)
    return _unpack_y(y)


# revision 14
# speedup vs baseline: 1.2157x; 1.2157x over previous
"""CondConv2d Trainium2 kernel (bf16 data path).

Data-parallel over batch: 32 samples -> 8 cores x 4 samples.
Per core:
  - x arrives host-prepadded in bf16 ([128, 2, XCOLS] per sample, zero
    margins), one contiguous DMA per sample; the expert table arrives
    bf16 and stays SBUF-resident (8 x [128, 4608]).
  - DMA order: consts, xpad[0], wtab[0..7], xpad[1..3] — minimizes the
    time until kgen[0] (which needs the full table) is buildable.
  - per-sample attention: pooled sums reduce over the padded bf16 image
    (co=0 on DVE, co=1 on Pool; pad zeros don't change the sum), tiny
    per-sample MLP + softmax + PE broadcast.  MLP matmuls for samples
    1..3 are interleaved between conv chunks so the in-order PE never
    stalls on late pooled sums.
  - per-sample kernels K_s = sum_w att[s,w] * W_w: column-split between
    DVE (tensor_scalar_mul 4x mode + tensor_tensor add 2x mode) and
    Pool (fused scalar_tensor_tensor), ~16us per sample, sample-major
    so conv for sample 0 starts as soon as kgen[0] is ready.
  - 3x3 conv as 18-matmul PSUM accumulation groups (2 cin chunks x 9
    taps) in bf16 (1 cycle/row vs 4 for fp32), bias added on ScalarE
    during the PSUM->SBUF copy, bf16 output DMA'd (issued from the
    Activation engine's DGE to keep SP free) as full padded rows;
    host strips pad cols and upcasts to fp32.
"""

import numpy as np

B, CIN, H, W = 32, 256, 56, 56
COUT, KH, KW = 256, 3, 3
ATT_IN, NW = 512, 8
NCORES = 8
BS = B // NCORES  # 4

WP = W + 1                 # 57: padded row width (56 + shared pad col; row
                           # h's pad doubles as row h+1's left pad)
PCOLS = H * WP             # 3192
MARG = WP + 1              # 58: left/right margin so tap windows stay in-bounds
XCOLS = MARG + PCOLS + MARG  # 3308
RT = 8                     # output rows per PSUM tile
NT = H // RT               # 7
NFREE = RT * WP            # 456 (<= 512 fp32 PSUM bank)
KFREE = 2 * 9 * COUT       # 4608 free cols of a generated kernel

_nc_cache = {}


def _split_waits(nc, max_inline=1):
    """Hoist inline sem waits beyond max_inline into standalone NoOps.

    This walrus build rejects instructions carrying more than one inline
    sync wait ("Too many sync wait commands"); standalone single-wait
    NoOps on the same engine are semantically identical and compile.
    """
    import copy

    import concourse.mybir as mybir

    cnt = 0
    for f in nc.m.functions:
        new_blocks = []
        for blk in f.blocks:
            out = []
            for inst in blk.instructions:
                si = inst.sync_info
                waits = list(si.on_wait) if si is not None else []
                if len(waits) > max_inline:
                    keep = waits[-max_inline:]
                    hoist = waits[:-max_inline]
                    for w in hoist:
                        nop = mybir.InstNoOp(name=f"WSPL-{cnt}", ins=[], outs=[])
                        cnt += 1
                        nop.engine = inst.engine
                        nop.sync_info = mybir.SyncInfo(on_wait=[w], on_update=[])
                        out.append(nop)
                    inst.sync_info = mybir.SyncInfo(
                        on_wait=keep, on_update=list(si.on_update)
                    )
                out.append(inst)
            new_blocks.append(copy.replace(blk, instructions=out))
        try:
            f.blocks[:] = new_blocks
        except TypeError:
            for i, nb in enumerate(new_blocks):
                f.blocks[i] = nb
    return cnt


def _build_nc(reps=1, split_waits=True):
    from contextlib import ExitStack

    import concourse.bass as bass
    import concourse.mybir as mybir
    import concourse.tile as tile

    fp32 = mybir.dt.float32
    bf16 = mybir.dt.bfloat16
    AF = mybir.ActivationFunctionType
    ALU = mybir.AluOpType
    AX = mybir.AxisListType

    nc = bass.Bass()

    # cblob packs the [128, ...] fp32 constants: attxT(16) w1t(1024)
    # w2t(512) wat(16) wb(16) -> 1584 cols
    cblob_d = nc.declare_dram_parameter("cblob", [128, 1584], fp32, isOutput=False)
    b12_d = nc.declare_dram_parameter("b12", [1, 256], fp32, isOutput=False)
    batt_d = nc.declare_dram_parameter("batt", [1, NW], fp32, isOutput=False)
    xpad_d = nc.declare_dram_parameter("xpad", [BS, 128, 2 * XCOLS], bf16, isOutput=False)
    wrep_d = nc.declare_dram_parameter("wrep", [NW, 128, KFREE], bf16, isOutput=False)
    y_d = nc.declare_dram_parameter("y", [BS, COUT, NT, NFREE], bf16, isOutput=True)

    with ExitStack() as ctx:
        tc = ctx.enter_context(tile.TileContext(nc))
        constp = ctx.enter_context(tc.tile_pool(name="const", bufs=1))
        wtabp = ctx.enter_context(tc.tile_pool(name="wtab", bufs=1))
        kgenp = ctx.enter_context(tc.tile_pool(name="kgen", bufs=1))
        xpadp = ctx.enter_context(tc.tile_pool(name="xpad", bufs=1))
        tmpp = ctx.enter_context(tc.tile_pool(name="tmp", bufs=2))
        osbp = ctx.enter_context(tc.tile_pool(name="osb", bufs=4))
        psum_s = ctx.enter_context(tc.tile_pool(name="psum_s", bufs=2, space="PSUM"))
        psum_c = ctx.enter_context(tc.tile_pool(name="psum_c", bufs=6, space="PSUM"))

        # --- constants (loaded once, reused across reps)
        # consts ride the Activation engine's DGE so SP's queue starts on
        # xpad/wtab (the critical path) immediately
        cblob = constp.tile([128, 1584], fp32)
        nc.scalar.dma_start(cblob[:], cblob_d[:])
        b12 = constp.tile([1, 256], fp32)
        nc.scalar.dma_start(b12[:], b12_d[:])
        batt = constp.tile([1, NW], fp32)
        nc.scalar.dma_start(batt[:], batt_d[:])
        attxT = cblob[:, 0:16].rearrange("p (k s) -> p k s", k=4)
        w1t = cblob[:, 16:1040].rearrange("p (k m) -> p k m", k=4)
        w2t = cblob[:, 1040:1552].rearrange("p (k m) -> p k m", k=2)
        wat = cblob[:, 1552:1568].rearrange("p (k m) -> p k m", k=2)
        wb = cblob[:, 1568:1584].rearrange("p (k m) -> p k m", k=2)
        ones11 = constp.tile([1, 1], fp32)
        nc.vector.memset(ones11[:], 1.0)
        ones_bc = constp.tile([1, 128], fp32)
        nc.vector.memset(ones_bc[:], 1.0)

        # --- persistent state
        kgen = [kgenp.tile([128, KFREE], bf16, tag=f"kgen{s}", name=f"kgen{s}") for s in range(BS)]
        xpad = [
            xpadp.tile([128, 2, XCOLS], bf16, tag=f"xpad{s}", name=f"xpad{s}")
            for s in range(BS)
        ]
        wtab = [
            wtabp.tile([128, KFREE], bf16, tag=f"wtab{w}", name=f"wtab{w}")
            for w in range(NW)
        ]
        pooledT = constp.tile([128, 2, BS], fp32)
        mixT = constp.tile([128, 2, BS], fp32)
        attbc = constp.tile([128, BS * NW], fp32)
        biasall = constp.tile([128, 2, BS], fp32)
        actsink = constp.tile([128, XCOLS], bf16)
        p3s = constp.tile([1, NW], fp32)
        mx = constp.tile([1, 1], fp32)
        nmx = constp.tile([1, 1], fp32)
        ex = constp.tile([1, NW], fp32)
        sm = constp.tile([1, 1], fp32)
        rcp = constp.tile([1, 1], fp32)
        att1 = constp.tile([1, NW], fp32)

        def att_ap(s, w):
            return attbc[:, s * NW + w : s * NW + w + 1]

        def issue_dmas(r):
            # SP issue order = critical-path order: xpad[0], xpad[1] (first
            # two attention chains), the full expert table, then the rest
            for s in (0, 1):
                nc.sync.dma_start(
                    xpad[s][:], xpad_d[s].rearrange("p (c q) -> p c q", c=2)
                )
            for w in range(NW):
                nc.sync.dma_start(wtab[w][:], wrep_d[w])
            for s in (2, 3):
                nc.sync.dma_start(
                    xpad[s][:], xpad_d[s].rearrange("p (c q) -> p c q", c=2)
                )

        if True:
            def chain_stage1(r, s):
                """DMA issue, pooled sums (both halves on ACT via accum_out,
                keeping the DVE queue clear for kgen FMAs), MLP layer 1."""
                if s == 0:
                    issue_dmas(r)
                for co in range(2):
                    nc.scalar.activation(
                        actsink[:], xpad[s][:, co, :], AF.Copy,
                        accum_out=pooledT[:, co : co + 1, s : s + 1],
                    )
                for c2 in range(2):
                    pm = psum_s.tile([128, 1], fp32, tag="mlps", name="pm")
                    nc.tensor.matmul(
                        pm[:], w1t[:, 0, c2 * 128 : (c2 + 1) * 128],
                        attxT[:, 0, s : s + 1], start=True, stop=False,
                    )
                    for ko in range(1, 4):
                        nc.tensor.matmul(
                            pm[:], w1t[:, ko, c2 * 128 : (c2 + 1) * 128],
                            attxT[:, ko, s : s + 1], start=False, stop=False,
                        )
                    for jo in range(2):
                        nc.tensor.matmul(
                            pm[:], w2t[:, jo, c2 * 128 : (c2 + 1) * 128],
                            pooledT[:, jo, s : s + 1], start=False, stop=False,
                        )
                    nc.tensor.matmul(
                        pm[:], b12[0:1, c2 * 128 : (c2 + 1) * 128], ones11[0:1, :],
                        start=False, stop=True,
                    )
                    nc.scalar.activation(mixT[:, c2, s : s + 1], pm[:], AF.Tanh)

            def chain_stage2(r, s):
                """Worker logits + softmax (tiny ops, partition 0)."""
                p3 = psum_s.tile([1, NW], fp32, tag="mlps", name="p3")
                nc.tensor.matmul(
                    p3[:], mixT[:, 0, s : s + 1], wat[:, 0, :], start=True, stop=False
                )
                nc.tensor.matmul(
                    p3[:], mixT[:, 1, s : s + 1], wat[:, 1, :], start=False, stop=False
                )
                nc.tensor.matmul(
                    p3[:], ones11[0:1, :], batt[0:1, :], start=False, stop=True
                )
                nc.vector.tensor_copy(p3s[:], p3[:])
                nc.vector.reduce_max(mx[:], p3s[:], axis=AX.X)
                nc.vector.tensor_scalar_mul(nmx[:], mx[:], -1.0)
                nc.scalar.activation(ex[:], p3s[:], AF.Exp, bias=nmx[:], scale=1.0)
                nc.vector.reduce_sum(sm[:], ex[:], axis=AX.X)
                nc.vector.reciprocal(rcp[:], sm[:])
                nc.vector.tensor_scalar_mul(att1[:], ex[:], rcp[:])

            def chain_stage3(r, s):
                """Broadcast attention, then kgen[s] + biasall[s] on DVE.

                tensor_scalar_mul runs in 4x mode and tensor_tensor add in
                2x mode for packed bf16 — together ~2.6x faster than the
                fused scalar_tensor_tensor chain (and walrus rejects
                TensorScalarPtr on Pool)."""
                pbc = psum_s.tile([128, NW], fp32, tag="mlps", name="pbc")
                nc.tensor.matmul(pbc[:], ones_bc[0:1, :], att1[0:1, :], start=True, stop=True)
                nc.vector.tensor_copy(attbc[:, s * NW : (s + 1) * NW], pbc[:])

                nc.vector.tensor_scalar_mul(
                    kgen[s][:], wtab[0][:], att_ap(s, 0)
                )
                for w in range(1, NW):
                    tmp = tmpp.tile([128, KFREE], bf16)
                    nc.vector.tensor_scalar_mul(
                        tmp[:], wtab[w][:], att_ap(s, w)
                    )
                    nc.vector.tensor_tensor(
                        kgen[s][:], kgen[s][:], tmp[:], ALU.add
                    )

                nc.vector.tensor_scalar_mul(
                    biasall[:, :, s : s + 1], wb[:, :, 0:1], att_ap(s, 0)
                )
                for w in range(1, NW):
                    nc.vector.scalar_tensor_tensor(
                        biasall[:, :, s : s + 1], wb[:, :, w : w + 1], att_ap(s, w),
                        biasall[:, :, s : s + 1], ALU.mult, ALU.add,
                    )

            def conv_chunk(s, c):
                """One cout-half of sample s: 7 PSUM groups of 18 matmuls."""
                kgv = kgen[s][:].rearrange("p (co t m) -> p co t m", co=2, t=9)
                xb = xpad[s]
                bias_ap = biasall[:, c, s : s + 1]
                for rt in range(NT):
                    p0 = rt * RT * WP
                    ps = psum_c.tile([128, NFREE], fp32)
                    k = 0
                    for co in range(2):
                        for t in range(9):
                            dh, dw = t // 3 - 1, t % 3 - 1
                            qs = MARG + p0 + dh * WP + dw
                            nc.tensor.matmul(
                                ps[:],
                                kgv[:, co, t, c * 128 : (c + 1) * 128],
                                xb[:, co, qs : qs + NFREE],
                                start=(k == 0), stop=(k == 17),
                            )
                            k += 1
                    ob = osbp.tile([128, NFREE], bf16)
                    nc.scalar.activation(
                        ob[:], ps[:], AF.Identity, bias=bias_ap, scale=1.0
                    )
                    nc.scalar.dma_start(
                        y_d[s, c * 128 : (c + 1) * 128, rt, :], ob[:]
                    )

            # software-pipelined schedule over all (rep, sample) units: each
            # unit's attention/kgen chain is spread over three conv-chunk
            # boundaries ahead of its own conv — across rep boundaries too —
            # so the in-order PE never stalls on a late attention chain,
            # cross-engine softmax round-trip, or table reload.
            seq = [(r, s) for r in range(reps) for s in range(BS)]
            N = len(seq)
            stages = {1: chain_stage1, 2: chain_stage2, 3: chain_stage3}
            events = [[] for _ in range(2 * N)]
            for k in range(N):
                for st, off in ((1, 2 * k - 4), (2, 2 * k - 3), (3, 2 * k - 2)):
                    events[max(0, off)].append((k, st))
            for b in range(2 * N):
                for k, st in events[b]:
                    stages[st](*seq[k])
                conv_chunk(seq[b // 2][1], b % 2)

    if split_waits:
        _split_waits(nc)
    return nc


def _prep_inputs(inputs):
    import ml_dtypes

    bf16 = ml_dtypes.bfloat16

    x = np.asarray(inputs["x"], np.float32)
    att_x = np.asarray(inputs["att_x"], np.float32)
    W_att1 = np.asarray(inputs["W_att1"], np.float32)
    b_att1 = np.asarray(inputs["b_att1"], np.float32)
    W_att2 = np.asarray(inputs["W_att2"], np.float32)
    b_att2 = np.asarray(inputs["b_att2"], np.float32)
    W_att = np.asarray(inputs["W_att"], np.float32)
    b_att = np.asarray(inputs["b_att"], np.float32)
    W_weight = np.asarray(inputs["W_weight"], np.float32)
    W_bias = np.asarray(inputs["W_bias"], np.float32)

    w1t = W_att1.T.reshape(4, 128, 256).transpose(1, 0, 2)  # [128, 4, 256]
    w2t = (W_att2 / float(H * W)).T.reshape(2, 128, 256).transpose(1, 0, 2)
    wat = W_att.T.reshape(2, 128, NW).transpose(1, 0, 2)
    wb = W_bias.reshape(2, 128, NW).transpose(1, 0, 2)
    b12 = np.ascontiguousarray((b_att1 + b_att2).reshape(1, 256))
    batt = np.ascontiguousarray(b_att.reshape(1, NW))
    # W_weight rows are [cout, cin, kh, kw] flattened; target [w, ci, (co2 t cout)]
    w5 = W_weight.reshape(COUT, 2, 128, 9, NW)
    wrep = np.ascontiguousarray(
        w5.transpose(4, 2, 1, 3, 0).reshape(NW, 128, KFREE)
    ).astype(bf16)

    # host-prepadded bf16 image: [B, 128 cin-part, 2 cin-chunk, XCOLS]
    xp = np.zeros((B, 128, 2, XCOLS), bf16)
    xv = xp[:, :, :, MARG : MARG + PCOLS].reshape(B, 128, 2, H, WP)[..., :W]
    xv[...] = x.reshape(B, 2, 128, H, W).transpose(0, 2, 1, 3, 4).astype(bf16)
    xp = xp.reshape(B, 128, 2 * XCOLS)

    in_maps = []
    for i in range(NCORES):
        axs = att_x[i * BS : (i + 1) * BS]  # [4, 512]
        attxT = axs.T.reshape(4, 128, BS).transpose(1, 0, 2)  # [128, 4, BS]
        cblob = np.concatenate(
            [
                attxT.reshape(128, 4 * BS)[:, : 4 * BS],
                w1t.reshape(128, 1024),
                w2t.reshape(128, 512),
                wat.reshape(128, 2 * NW),
                wb.reshape(128, 2 * NW),
            ],
            axis=1,
        ).astype(np.float32)
        in_maps.append(
            {
                "cblob": np.ascontiguousarray(cblob),
                "b12": b12,
                "batt": batt,
                "xpad": xp[i * BS : (i + 1) * BS],
                "wrep": wrep,
            }
        )
    return in_maps


def _unpack_y(y_bf16):
    """[B, COUT, NT, NFREE] bf16 padded rows -> [B, COUT, H, W] fp32."""
    y = np.asarray(y_bf16).reshape(B, COUT, NT, RT, WP)[..., 0:W]
    return np.ascontiguousarray(y.reshape(B, COUT, H, W).astype(np.float32))


def kernel(**inputs):
    from concourse.bass_utils import run_bass_kernel_spmd

    if "nc" not in _nc_cache:
        _nc_cache["nc"] = _build_nc()
    in_maps = _prep_inputs(inputs)
    res = run_bass_kernel_spmd(_nc_cache["nc"], in_maps, core_ids=list(range(NCORES)))
    y = np.concatenate([np.asarray(res.results[i]["y"]) for i in range(NCORES)], axis=0)
    return _unpack_y(y)


# revision 15
# speedup vs baseline: 1.2286x; 1.0106x over previous
"""CondConv2d Trainium2 kernel (bf16 data path).

Data-parallel over batch: 32 samples -> 8 cores x 4 samples.
Per core:
  - x arrives host-prepadded in bf16 ([128, 2, XCOLS] per sample, zero
    margins, 57-wide rows whose single pad column doubles as the next
    row's left pad), one contiguous DMA per sample; the expert table
    arrives bf16 and stays SBUF-resident (8 x [128, 4608]).
  - per-sample attention: pooled sums accumulate on the ACT engine
    (activation Copy + accum_out over the padded image; pad zeros don't
    change the sum), tiny per-sample MLP on the PE, softmax on DVE/ACT,
    attention row broadcast to all partitions via a rank-1 PE matmul.
  - per-sample kernels K_s = sum_w att[s,w] * W_w on the DVE as a
    tensor_scalar_mul (4x packed-bf16 mode) + tensor_tensor add (2x)
    chain — ~2.6x faster than fused scalar_tensor_tensor, and walrus
    rejects TensorScalarPtr on Pool.
  - 3x3 conv as 18-matmul PSUM accumulation groups (2 cin chunks x 9
    taps) in bf16 (1 cycle/row vs 4 for fp32), moving operand = shifted
    windows of the padded image, bias added on ScalarE during the
    PSUM->SBUF drain, bf16 output DMA'd (issued from the ACT engine's
    DGE to keep SP free) as full padded rows; host strips pad cols and
    upcasts to fp32.
  - the whole thing is software-pipelined: each (rep, sample) unit's
    attention/kgen chain is issued, in three stages, at conv-chunk
    boundaries 4/3/2 chunks ahead of its own conv, so the in-order PE
    runs the conv matmul stream without stalls (also across reps in the
    timing build).
"""

import numpy as np

B, CIN, H, W = 32, 256, 56, 56
COUT, KH, KW = 256, 3, 3
ATT_IN, NW = 512, 8
NCORES = 8
BS = B // NCORES  # 4

WP = W + 1                 # 57: padded row width (56 + shared pad col; row
                           # h's pad doubles as row h+1's left pad)
PCOLS = H * WP             # 3192
MARG = WP + 1              # 58: left/right margin so tap windows stay in-bounds
XCOLS = MARG + PCOLS + MARG  # 3308
RT = 8                     # output rows per PSUM tile
NT = H // RT               # 7
NFREE = RT * WP            # 456 (<= 512 fp32 PSUM bank)
KFREE = 2 * 9 * COUT       # 4608 free cols of a generated kernel

_nc_cache = {}


def _split_waits(nc, max_inline=1):
    """Hoist inline sem waits beyond max_inline into standalone NoOps.

    This walrus build rejects instructions carrying more than one inline
    sync wait ("Too many sync wait commands"); standalone single-wait
    NoOps on the same engine are semantically identical and compile.
    """
    import copy

    import concourse.mybir as mybir

    cnt = 0
    for f in nc.m.functions:
        new_blocks = []
        for blk in f.blocks:
            out = []
            for inst in blk.instructions:
                si = inst.sync_info
                waits = list(si.on_wait) if si is not None else []
                if len(waits) > max_inline:
                    keep = waits[-max_inline:]
                    hoist = waits[:-max_inline]
                    for w in hoist:
                        nop = mybir.InstNoOp(name=f"WSPL-{cnt}", ins=[], outs=[])
                        cnt += 1
                        nop.engine = inst.engine
                        nop.sync_info = mybir.SyncInfo(on_wait=[w], on_update=[])
                        out.append(nop)
                    inst.sync_info = mybir.SyncInfo(
                        on_wait=keep, on_update=list(si.on_update)
                    )
                out.append(inst)
            new_blocks.append(copy.replace(blk, instructions=out))
        try:
            f.blocks[:] = new_blocks
        except TypeError:
            for i, nb in enumerate(new_blocks):
                f.blocks[i] = nb
    return cnt


def _build_nc(reps=1, split_waits=True):
    from contextlib import ExitStack

    import concourse.bass as bass
    import concourse.mybir as mybir
    import concourse.tile as tile

    fp32 = mybir.dt.float32
    bf16 = mybir.dt.bfloat16
    AF = mybir.ActivationFunctionType
    ALU = mybir.AluOpType
    AX = mybir.AxisListType

    nc = bass.Bass()

    # cblob packs the [128, ...] fp32 constants: attxT(16) w1t(1024)
    # w2t(512) wat(16) wb(16) -> 1584 cols
    cblob_d = nc.declare_dram_parameter("cblob", [128, 1584], fp32, isOutput=False)
    b12_d = nc.declare_dram_parameter("b12", [1, 256], fp32, isOutput=False)
    batt_d = nc.declare_dram_parameter("batt", [1, NW], fp32, isOutput=False)
    xpad_d = nc.declare_dram_parameter("xpad", [BS, 128, 2 * XCOLS], bf16, isOutput=False)
    wrep_d = nc.declare_dram_parameter("wrep", [NW, 128, KFREE], bf16, isOutput=False)
    y_d = nc.declare_dram_parameter("y", [BS, COUT, NT, NFREE], bf16, isOutput=True)

    with ExitStack() as ctx:
        tc = ctx.enter_context(tile.TileContext(nc))
        constp = ctx.enter_context(tc.tile_pool(name="const", bufs=1))
        wtabp = ctx.enter_context(tc.tile_pool(name="wtab", bufs=1))
        kgenp = ctx.enter_context(tc.tile_pool(name="kgen", bufs=1))
        xpadp = ctx.enter_context(tc.tile_pool(name="xpad", bufs=1))
        tmpp = ctx.enter_context(tc.tile_pool(name="tmp", bufs=2))
        osbp = ctx.enter_context(tc.tile_pool(name="osb", bufs=4))
        psum_s = ctx.enter_context(tc.tile_pool(name="psum_s", bufs=2, space="PSUM"))
        psum_c = ctx.enter_context(tc.tile_pool(name="psum_c", bufs=6, space="PSUM"))

        # --- constants (loaded once, reused across reps)
        # consts ride the Activation engine's DGE so SP's queue starts on
        # xpad/wtab (the critical path) immediately
        cblob = constp.tile([128, 1584], fp32)
        nc.scalar.dma_start(cblob[:], cblob_d[:])
        b12 = constp.tile([1, 256], fp32)
        nc.scalar.dma_start(b12[:], b12_d[:])
        batt = constp.tile([1, NW], fp32)
        nc.scalar.dma_start(batt[:], batt_d[:])
        attxT = cblob[:, 0:16].rearrange("p (k s) -> p k s", k=4)
        w1t = cblob[:, 16:1040].rearrange("p (k m) -> p k m", k=4)
        w2t = cblob[:, 1040:1552].rearrange("p (k m) -> p k m", k=2)
        wat = cblob[:, 1552:1568].rearrange("p (k m) -> p k m", k=2)
        wb = cblob[:, 1568:1584].rearrange("p (k m) -> p k m", k=2)
        ones11 = constp.tile([1, 1], fp32)
        nc.vector.memset(ones11[:], 1.0)
        ones_bc = constp.tile([1, 128], fp32)
        nc.vector.memset(ones_bc[:], 1.0)

        # --- persistent state
        kgen = [kgenp.tile([128, KFREE], bf16, tag=f"kgen{s}", name=f"kgen{s}") for s in range(BS)]
        xpad = [
            xpadp.tile([128, 2, XCOLS], bf16, tag=f"xpad{s}", name=f"xpad{s}")
            for s in range(BS)
        ]
        wtab = [
            wtabp.tile([128, KFREE], bf16, tag=f"wtab{w}", name=f"wtab{w}")
            for w in range(NW)
        ]
        pooledT = constp.tile([128, 2, BS], fp32)
        mixT = constp.tile([128, 2, BS], fp32)
        attbc = constp.tile([128, BS * NW], fp32)
        biasall = constp.tile([128, 2, BS], fp32)
        actsink = constp.tile([128, XCOLS], bf16)
        p3s = constp.tile([1, NW], fp32)
        mx = constp.tile([1, 1], fp32)
        nmx = constp.tile([1, 1], fp32)
        ex = constp.tile([1, NW], fp32)
        sm = constp.tile([1, 1], fp32)
        rcp = constp.tile([1, 1], fp32)
        att1 = constp.tile([1, NW], fp32)

        def att_ap(s, w):
            return attbc[:, s * NW + w : s * NW + w + 1]

        def issue_dmas(r):
            # SP issue order = critical-path order: xpad[0], xpad[1] (first
            # two attention chains), the full expert table, then the rest
            for s in (0, 1):
                nc.sync.dma_start(
                    xpad[s][:], xpad_d[s].rearrange("p (c q) -> p c q", c=2)
                )
            for w in range(NW):
                nc.sync.dma_start(wtab[w][:], wrep_d[w])
            for s in (2, 3):
                nc.sync.dma_start(
                    xpad[s][:], xpad_d[s].rearrange("p (c q) -> p c q", c=2)
                )

        if True:
            def chain_stage1(r, s):
                """DMA issue, pooled sums (both halves on ACT via accum_out,
                keeping the DVE queue clear for kgen FMAs), MLP layer 1."""
                if s == 0:
                    issue_dmas(r)
                for co in range(2):
                    nc.scalar.activation(
                        actsink[:], xpad[s][:, co, :], AF.Copy,
                        accum_out=pooledT[:, co : co + 1, s : s + 1],
                    )
                for c2 in range(2):
                    pm = psum_s.tile([128, 1], fp32, tag="mlps", name="pm")
                    nc.tensor.matmul(
                        pm[:], w1t[:, 0, c2 * 128 : (c2 + 1) * 128],
                        attxT[:, 0, s : s + 1], start=True, stop=False,
                    )
                    for ko in range(1, 4):
                        nc.tensor.matmul(
                            pm[:], w1t[:, ko, c2 * 128 : (c2 + 1) * 128],
                            attxT[:, ko, s : s + 1], start=False, stop=False,
                        )
                    for jo in range(2):
                        nc.tensor.matmul(
                            pm[:], w2t[:, jo, c2 * 128 : (c2 + 1) * 128],
                            pooledT[:, jo, s : s + 1], start=False, stop=False,
                        )
                    nc.tensor.matmul(
                        pm[:], b12[0:1, c2 * 128 : (c2 + 1) * 128], ones11[0:1, :],
                        start=False, stop=True,
                    )
                    nc.scalar.activation(mixT[:, c2, s : s + 1], pm[:], AF.Tanh)

            def chain_stage2(r, s):
                """Worker logits + softmax (tiny ops, partition 0)."""
                p3 = psum_s.tile([1, NW], fp32, tag="mlps", name="p3")
                nc.tensor.matmul(
                    p3[:], mixT[:, 0, s : s + 1], wat[:, 0, :], start=True, stop=False
                )
                nc.tensor.matmul(
                    p3[:], mixT[:, 1, s : s + 1], wat[:, 1, :], start=False, stop=False
                )
                nc.tensor.matmul(
                    p3[:], ones11[0:1, :], batt[0:1, :], start=False, stop=True
                )
                nc.vector.tensor_copy(p3s[:], p3[:])
                nc.vector.reduce_max(mx[:], p3s[:], axis=AX.X)
                nc.vector.tensor_scalar_mul(nmx[:], mx[:], -1.0)
                nc.scalar.activation(ex[:], p3s[:], AF.Exp, bias=nmx[:], scale=1.0)
                nc.vector.reduce_sum(sm[:], ex[:], axis=AX.X)
                nc.vector.reciprocal(rcp[:], sm[:])
                nc.vector.tensor_scalar_mul(att1[:], ex[:], rcp[:])

            def chain_stage3(r, s):
                """Broadcast attention, then kgen[s] + biasall[s] on DVE.

                tensor_scalar_mul runs in 4x mode and tensor_tensor add in
                2x mode for packed bf16 — together ~2.6x faster than the
                fused scalar_tensor_tensor chain (and walrus rejects
                TensorScalarPtr on Pool)."""
                pbc = psum_s.tile([128, NW], fp32, tag="mlps", name="pbc")
                nc.tensor.matmul(pbc[:], ones_bc[0:1, :], att1[0:1, :], start=True, stop=True)
                nc.vector.tensor_copy(attbc[:, s * NW : (s + 1) * NW], pbc[:])

                nc.vector.tensor_scalar_mul(
                    kgen[s][:], wtab[0][:], att_ap(s, 0)
                )
                for w in range(1, NW):
                    tmp = tmpp.tile([128, KFREE], bf16)
                    nc.vector.tensor_scalar_mul(
                        tmp[:], wtab[w][:], att_ap(s, w)
                    )
                    nc.vector.tensor_tensor(
                        kgen[s][:], kgen[s][:], tmp[:], ALU.add
                    )

                nc.vector.tensor_scalar_mul(
                    biasall[:, :, s : s + 1], wb[:, :, 0:1], att_ap(s, 0)
                )
                for w in range(1, NW):
                    nc.vector.scalar_tensor_tensor(
                        biasall[:, :, s : s + 1], wb[:, :, w : w + 1], att_ap(s, w),
                        biasall[:, :, s : s + 1], ALU.mult, ALU.add,
                    )

            def conv_chunk(s, c):
                """One cout-half of sample s: 7 PSUM groups of 18 matmuls."""
                kgv = kgen[s][:].rearrange("p (co t m) -> p co t m", co=2, t=9)
                xb = xpad[s]
                bias_ap = biasall[:, c, s : s + 1]
                for rt in range(NT):
                    p0 = rt * RT * WP
                    ps = psum_c.tile([128, NFREE], fp32)
                    k = 0
                    for co in range(2):
                        for t in range(9):
                            dh, dw = t // 3 - 1, t % 3 - 1
                            qs = MARG + p0 + dh * WP + dw
                            nc.tensor.matmul(
                                ps[:],
                                kgv[:, co, t, c * 128 : (c + 1) * 128],
                                xb[:, co, qs : qs + NFREE],
                                start=(k == 0), stop=(k == 17),
                            )
                            k += 1
                    ob = osbp.tile([128, NFREE], bf16)
                    nc.scalar.activation(
                        ob[:], ps[:], AF.Identity, bias=bias_ap, scale=1.0
                    )
                    nc.scalar.dma_start(
                        y_d[s, c * 128 : (c + 1) * 128, rt, :], ob[:]
                    )

            # software-pipelined schedule over all (rep, sample) units: each
            # unit's attention/kgen chain is spread over three conv-chunk
            # boundaries ahead of its own conv — across rep boundaries too —
            # so the in-order PE never stalls on a late attention chain,
            # cross-engine softmax round-trip, or table reload.
            seq = [(r, s) for r in range(reps) for s in range(BS)]
            N = len(seq)
            stages = {1: chain_stage1, 2: chain_stage2, 3: chain_stage3}
            events = [[] for _ in range(2 * N)]
            for k in range(N):
                for st, off in ((1, 2 * k - 4), (2, 2 * k - 3), (3, 2 * k - 2)):
                    events[max(0, off)].append((k, st))
            for b in range(2 * N):
                for k, st in events[b]:
                    stages[st](*seq[k])
                conv_chunk(seq[b // 2][1], b % 2)

    if split_waits:
        _split_waits(nc)
    return nc


def _prep_inputs(inputs):
    import ml_dtypes

    bf16 = ml_dtypes.bfloat16

    x = np.asarray(inputs["x"], np.float32)
    att_x = np.asarray(inputs["att_x"], np.float32)
    W_att1 = np.asarray(inputs["W_att1"], np.float32)
    b_att1 = np.asarray(inputs["b_att1"], np.float32)
    W_att2 = np.asarray(inputs["W_att2"], np.float32)
    b_att2 = np.asarray(inputs["b_att2"], np.float32)
    W_att = np.asarray(inputs["W_att"], np.float32)
    b_att = np.asarray(inputs["b_att"], np.float32)
    W_weight = np.asarray(inputs["W_weight"], np.float32)
    W_bias = np.asarray(inputs["W_bias"], np.float32)

    w1t = W_att1.T.reshape(4, 128, 256).transpose(1, 0, 2)  # [128, 4, 256]
    w2t = (W_att2 / float(H * W)).T.reshape(2, 128, 256).transpose(1, 0, 2)
    wat = W_att.T.reshape(2, 128, NW).transpose(1, 0, 2)
    wb = W_bias.reshape(2, 128, NW).transpose(1, 0, 2)
    b12 = np.ascontiguousarray((b_att1 + b_att2).reshape(1, 256))
    batt = np.ascontiguousarray(b_att.reshape(1, NW))
    # W_weight rows are [cout, cin, kh, kw] flattened; target [w, ci, (co2 t cout)]
    w5 = W_weight.reshape(COUT, 2, 128, 9, NW)
    wrep = np.ascontiguousarray(
        w5.transpose(4, 2, 1, 3, 0).reshape(NW, 128, KFREE)
    ).astype(bf16)

    # host-prepadded bf16 image: [B, 128 cin-part, 2 cin-chunk, XCOLS]
    xp = np.zeros((B, 128, 2, XCOLS), bf16)
    xv = xp[:, :, :, MARG : MARG + PCOLS].reshape(B, 128, 2, H, WP)[..., :W]
    xv[...] = x.reshape(B, 2, 128, H, W).transpose(0, 2, 1, 3, 4).astype(bf16)
    xp = xp.reshape(B, 128, 2 * XCOLS)

    in_maps = []
    for i in range(NCORES):
        axs = att_x[i * BS : (i + 1) * BS]  # [4, 512]
        attxT = axs.T.reshape(4, 128, BS).transpose(1, 0, 2)  # [128, 4, BS]
        cblob = np.concatenate(
            [
                attxT.reshape(128, 4 * BS)[:, : 4 * BS],
                w1t.reshape(128, 1024),
                w2t.reshape(128, 512),
                wat.reshape(128, 2 * NW),
                wb.reshape(128, 2 * NW),
            ],
            axis=1,
        ).astype(np.float32)
        in_maps.append(
            {
                "cblob": np.ascontiguousarray(cblob),
                "b12": b12,
                "batt": batt,
                "xpad": xp[i * BS : (i + 1) * BS],
                "wrep": wrep,
            }
        )
    return in_maps


def _unpack_y(y_bf16):
    """[B, COUT, NT, NFREE] bf16 padded rows -> [B, COUT, H, W] fp32."""
    y = np.asarray(y_bf16).reshape(B, COUT, NT, RT, WP)[..., 0:W]
    return np.ascontiguousarray(y.reshape(B, COUT, H, W).astype(np.float32))


def kernel(**inputs):
    from concourse.bass_utils import run_bass_kernel_spmd

    if "nc" not in _nc_cache:
        _nc_cache["nc"] = _build_nc()
    in_maps = _prep_inputs(inputs)
    res = run_bass_kernel_spmd(_nc_cache["nc"], in_maps, core_ids=list(range(NCORES)))
    y = np.concatenate([np.asarray(res.results[i]["y"]) for i in range(NCORES)], axis=0)
    return _unpack_y(y)


# revision 18
# speedup vs baseline: 1.3359x; 1.0873x over previous
"""CondConv2d Trainium2 kernel (bf16 data path).

Data-parallel over batch: 32 samples -> 8 cores x 4 samples.
Per core:
  - x arrives host-prepadded in bf16 ([128, 2, XCOLS] per sample, zero
    margins, 57-wide rows whose single pad column doubles as the next
    row's left pad), one contiguous DMA per sample; the expert table
    arrives bf16 and stays SBUF-resident (8 x [128, 4608]).
  - per-sample attention: pooled sums reduce on the DVE over the padded
    image (pad zeros don't change the sum), tiny per-sample MLP on the
    PE, softmax on DVE/ACT, attention row broadcast to all partitions
    via a rank-1 PE matmul.
  - per-sample kernels K_s = sum_w att[s,w] * W_w on the DVE as a
    tensor_scalar_mul (4x packed-bf16 mode) + tensor_tensor add (2x)
    chain — ~2.6x faster than fused scalar_tensor_tensor, and walrus
    rejects TensorScalarPtr on Pool.
  - 3x3 conv as 18-matmul PSUM accumulation groups (2 cin chunks x 9
    taps) in bf16 (1 cycle/row vs 4 for fp32), moving operand = shifted
    windows of the padded image, bias added on ScalarE during the
    PSUM->SBUF drain, bf16 output DMA'd (issued from the ACT engine's
    DGE to keep SP free) as full padded rows; host strips pad cols and
    upcasts to fp32.
  - the whole thing is software-pipelined: each (rep, sample) unit's
    attention/kgen chain is issued, in three stages, at conv-chunk
    boundaries 4/3/2 chunks ahead of its own conv, so the in-order PE
    runs the conv matmul stream without stalls (also across reps in the
    timing build).
"""

import numpy as np

B, CIN, H, W = 32, 256, 56, 56
COUT, KH, KW = 256, 3, 3
ATT_IN, NW = 512, 8
NCORES = 8
BS = B // NCORES  # 4

WP = W + 1                 # 57: padded row width (56 + shared pad col; row
                           # h's pad doubles as row h+1's left pad)
PCOLS = H * WP             # 3192
MARG = WP + 1              # 58: left/right margin so tap windows stay in-bounds
XCOLS = MARG + PCOLS + MARG  # 3308
RT = 8                     # output rows per PSUM tile
NT = H // RT               # 7
NFREE = RT * WP            # 456 (<= 512 fp32 PSUM bank)
KFREE = 2 * 9 * COUT       # 4608 free cols of a generated kernel

_nc_cache = {}


def _split_waits(nc, max_inline=1):
    """Hoist inline sem waits beyond max_inline into standalone NoOps.

    This walrus build rejects instructions carrying more than one inline
    sync wait ("Too many sync wait commands"); standalone single-wait
    NoOps on the same engine are semantically identical and compile.
    """
    import copy

    import concourse.mybir as mybir

    cnt = 0
    for f in nc.m.functions:
        new_blocks = []
        for blk in f.blocks:
            out = []
            for inst in blk.instructions:
                si = inst.sync_info
                waits = list(si.on_wait) if si is not None else []
                if len(waits) > max_inline:
                    keep = waits[-max_inline:]
                    hoist = waits[:-max_inline]
                    for w in hoist:
                        nop = mybir.InstNoOp(name=f"WSPL-{cnt}", ins=[], outs=[])
                        cnt += 1
                        nop.engine = inst.engine
                        nop.sync_info = mybir.SyncInfo(on_wait=[w], on_update=[])
                        out.append(nop)
                    inst.sync_info = mybir.SyncInfo(
                        on_wait=keep, on_update=list(si.on_update)
                    )
                out.append(inst)
            new_blocks.append(copy.replace(blk, instructions=out))
        try:
            f.blocks[:] = new_blocks
        except TypeError:
            for i, nb in enumerate(new_blocks):
                f.blocks[i] = nb
    return cnt


def _build_nc(reps=1, split_waits=True):
    from contextlib import ExitStack

    import concourse.bass as bass
    import concourse.mybir as mybir
    import concourse.tile as tile

    fp32 = mybir.dt.float32
    bf16 = mybir.dt.bfloat16
    AF = mybir.ActivationFunctionType
    ALU = mybir.AluOpType
    AX = mybir.AxisListType

    nc = bass.Bass()

    # cblob packs the [128, ...] fp32 constants: attxT(16) w1t(1024)
    # w2t(512) wat(16) wb(16) -> 1584 cols
    cblob_d = nc.declare_dram_parameter("cblob", [128, 1584], fp32, isOutput=False)
    b12_d = nc.declare_dram_parameter("b12", [1, 256], fp32, isOutput=False)
    batt_d = nc.declare_dram_parameter("batt", [1, NW], fp32, isOutput=False)
    xpad_d = nc.declare_dram_parameter("xpad", [BS, 128, 2 * XCOLS], bf16, isOutput=False)
    wrep_d = nc.declare_dram_parameter("wrep", [NW, 128, KFREE], bf16, isOutput=False)
    y_d = nc.declare_dram_parameter("y", [BS, COUT, NT, NFREE], bf16, isOutput=True)

    with ExitStack() as ctx:
        tc = ctx.enter_context(tile.TileContext(nc))
        constp = ctx.enter_context(tc.tile_pool(name="const", bufs=1))
        wtabp = ctx.enter_context(tc.tile_pool(name="wtab", bufs=1))
        kgenp = ctx.enter_context(tc.tile_pool(name="kgen", bufs=1))
        xpadp = ctx.enter_context(tc.tile_pool(name="xpad", bufs=1))
        tmpp = ctx.enter_context(tc.tile_pool(name="tmp", bufs=2))
        osbp = ctx.enter_context(tc.tile_pool(name="osb", bufs=4))
        psum_s = ctx.enter_context(tc.tile_pool(name="psum_s", bufs=2, space="PSUM"))
        psum_c = ctx.enter_context(tc.tile_pool(name="psum_c", bufs=6, space="PSUM"))

        # --- constants (loaded once, reused across reps)
        # consts ride the Activation engine's DGE so SP's queue starts on
        # xpad/wtab (the critical path) immediately
        cblob = constp.tile([128, 1584], fp32)
        nc.scalar.dma_start(cblob[:], cblob_d[:])
        b12 = constp.tile([1, 256], fp32)
        nc.scalar.dma_start(b12[:], b12_d[:])
        batt = constp.tile([1, NW], fp32)
        nc.scalar.dma_start(batt[:], batt_d[:])
        attxT = cblob[:, 0:16].rearrange("p (k s) -> p k s", k=4)
        w1t = cblob[:, 16:1040].rearrange("p (k m) -> p k m", k=4)
        w2t = cblob[:, 1040:1552].rearrange("p (k m) -> p k m", k=2)
        wat = cblob[:, 1552:1568].rearrange("p (k m) -> p k m", k=2)
        wb = cblob[:, 1568:1584].rearrange("p (k m) -> p k m", k=2)
        ones11 = constp.tile([1, 1], fp32)
        nc.vector.memset(ones11[:], 1.0)
        ones_bc = constp.tile([1, 128], fp32)
        nc.vector.memset(ones_bc[:], 1.0)

        # --- persistent state
        kgen = [kgenp.tile([128, KFREE], bf16, tag=f"kgen{s}", name=f"kgen{s}") for s in range(BS)]
        xpad = [
            xpadp.tile([128, 2, XCOLS], bf16, tag=f"xpad{s}", name=f"xpad{s}")
            for s in range(BS)
        ]
        wtab = [
            wtabp.tile([128, KFREE], bf16, tag=f"wtab{w}", name=f"wtab{w}")
            for w in range(NW)
        ]
        pooledT = constp.tile([128, 2, BS], fp32)
        mixT = constp.tile([128, 2, BS], fp32)
        attbc = constp.tile([128, BS * NW], fp32)
        biasall = constp.tile([128, 2, BS], fp32)
        p3s = constp.tile([1, NW], fp32)
        mx = constp.tile([1, 1], fp32)
        nmx = constp.tile([1, 1], fp32)
        ex = constp.tile([1, NW], fp32)
        sm = constp.tile([1, 1], fp32)
        rcp = constp.tile([1, 1], fp32)
        att1 = constp.tile([1, NW], fp32)

        def att_ap(s, w):
            return attbc[:, s * NW + w : s * NW + w + 1]

        def issue_dmas(r):
            # SP issue order = critical-path order: xpad[0], xpad[1] (first
            # two attention chains), the full expert table, then the rest
            for s in (0, 1):
                nc.sync.dma_start(
                    xpad[s][:], xpad_d[s].rearrange("p (c q) -> p c q", c=2)
                )
            for w in range(NW):
                nc.sync.dma_start(wtab[w][:], wrep_d[w])
            for s in (2, 3):
                nc.sync.dma_start(
                    xpad[s][:], xpad_d[s].rearrange("p (c q) -> p c q", c=2)
                )

        if True:
            def chain_stage1(r, s):
                """DMA issue, pooled sums (both halves on ACT via accum_out,
                keeping the DVE queue clear for kgen FMAs), MLP layer 1."""
                if s == 0:
                    issue_dmas(r)
                # pooled sums on DVE: keeps the ACT engine's instruction
                # stream (PSUM drains) free of Copy-table switches
                for co in range(2):
                    nc.vector.reduce_sum(
                        pooledT[:, co : co + 1, s : s + 1],
                        xpad[s][:, co : co + 1, :], axis=AX.X,
                    )
                for c2 in range(2):
                    pm = psum_s.tile([128, 1], fp32, tag="mlps", name="pm")
                    nc.tensor.matmul(
                        pm[:], w1t[:, 0, c2 * 128 : (c2 + 1) * 128],
                        attxT[:, 0, s : s + 1], start=True, stop=False,
                    )
                    for ko in range(1, 4):
                        nc.tensor.matmul(
                            pm[:], w1t[:, ko, c2 * 128 : (c2 + 1) * 128],
                            attxT[:, ko, s : s + 1], start=False, stop=False,
                        )
                    for jo in range(2):
                        nc.tensor.matmul(
                            pm[:], w2t[:, jo, c2 * 128 : (c2 + 1) * 128],
                            pooledT[:, jo, s : s + 1], start=False, stop=False,
                        )
                    nc.tensor.matmul(
                        pm[:], b12[0:1, c2 * 128 : (c2 + 1) * 128], ones11[0:1, :],
                        start=False, stop=True,
                    )
                    nc.scalar.activation(mixT[:, c2, s : s + 1], pm[:], AF.Tanh)

            def chain_stage2(r, s):
                """Worker logits + softmax (tiny ops, partition 0)."""
                p3 = psum_s.tile([1, NW], fp32, tag="mlps", name="p3")
                nc.tensor.matmul(
                    p3[:], mixT[:, 0, s : s + 1], wat[:, 0, :], start=True, stop=False
                )
                nc.tensor.matmul(
                    p3[:], mixT[:, 1, s : s + 1], wat[:, 1, :], start=False, stop=False
                )
                nc.tensor.matmul(
                    p3[:], ones11[0:1, :], batt[0:1, :], start=False, stop=True
                )
                nc.vector.tensor_copy(p3s[:], p3[:])
                nc.vector.reduce_max(mx[:], p3s[:], axis=AX.X)
                nc.vector.tensor_scalar_mul(nmx[:], mx[:], -1.0)
                nc.scalar.activation(ex[:], p3s[:], AF.Exp, bias=nmx[:], scale=1.0)
                nc.vector.reduce_sum(sm[:], ex[:], axis=AX.X)
                nc.vector.reciprocal(rcp[:], sm[:])
                nc.vector.tensor_scalar_mul(att1[:], ex[:], rcp[:])

            def chain_stage3(r, s):
                """Broadcast attention, then kgen[s] + biasall[s] on DVE.

                tensor_scalar_mul runs in 4x mode and tensor_tensor add in
                2x mode for packed bf16 — together ~2.6x faster than the
                fused scalar_tensor_tensor chain (and walrus rejects
                TensorScalarPtr on Pool)."""
                pbc = psum_s.tile([128, NW], fp32, tag="mlps", name="pbc")
                nc.tensor.matmul(pbc[:], ones_bc[0:1, :], att1[0:1, :], start=True, stop=True)
                nc.vector.tensor_copy(attbc[:, s * NW : (s + 1) * NW], pbc[:])

                nc.vector.tensor_scalar_mul(
                    kgen[s][:], wtab[0][:], att_ap(s, 0)
                )
                for w in range(1, NW):
                    tmp = tmpp.tile([128, KFREE], bf16)
                    nc.vector.tensor_scalar_mul(
                        tmp[:], wtab[w][:], att_ap(s, w)
                    )
                    nc.vector.tensor_tensor(
                        kgen[s][:], kgen[s][:], tmp[:], ALU.add
                    )

                nc.vector.tensor_scalar_mul(
                    biasall[:, :, s : s + 1], wb[:, :, 0:1], att_ap(s, 0)
                )
                for w in range(1, NW):
                    nc.vector.scalar_tensor_tensor(
                        biasall[:, :, s : s + 1], wb[:, :, w : w + 1], att_ap(s, w),
                        biasall[:, :, s : s + 1], ALU.mult, ALU.add,
                    )

            def conv_chunk(s, c):
                """One cout-half of sample s: 7 PSUM groups of 18 matmuls."""
                kgv = kgen[s][:].rearrange("p (co t m) -> p co t m", co=2, t=9)
                xb = xpad[s]
                bias_ap = biasall[:, c, s : s + 1]
                for rt in range(NT):
                    p0 = rt * RT * WP
                    ps = psum_c.tile([128, NFREE], fp32)
                    k = 0
                    for co in range(2):
                        for t in range(9):
                            dh, dw = t // 3 - 1, t % 3 - 1
                            qs = MARG + p0 + dh * WP + dw
                            nc.tensor.matmul(
                                ps[:],
                                kgv[:, co, t, c * 128 : (c + 1) * 128],
                                xb[:, co, qs : qs + NFREE],
                                start=(k == 0), stop=(k == 17),
                            )
                            k += 1
                    ob = osbp.tile([128, NFREE], bf16)
                    nc.scalar.activation(
                        ob[:], ps[:], AF.Identity, bias=bias_ap, scale=1.0
                    )
                    nc.scalar.dma_start(
                        y_d[s, c * 128 : (c + 1) * 128, rt, :], ob[:]
                    )

            # software-pipelined schedule over all (rep, sample) units: each
            # unit's attention/kgen chain is spread over three conv-chunk
            # boundaries ahead of its own conv — across rep boundaries too —
            # so the in-order PE never stalls on a late attention chain,
            # cross-engine softmax round-trip, or table reload.
            seq = [(r, s) for r in range(reps) for s in range(BS)]
            N = len(seq)
            stages = {1: chain_stage1, 2: chain_stage2, 3: chain_stage3}
            events = [[] for _ in range(2 * N)]
            for k in range(N):
                for st, off in ((1, 2 * k - 4), (2, 2 * k - 3), (3, 2 * k - 2)):
                    events[max(0, off)].append((k, st))
            for b in range(2 * N):
                for k, st in events[b]:
                    stages[st](*seq[k])
                conv_chunk(seq[b // 2][1], b % 2)

    if split_waits:
        _split_waits(nc)
    return nc


def _prep_inputs(inputs):
    import ml_dtypes

    bf16 = ml_dtypes.bfloat16

    x = np.asarray(inputs["x"], np.float32)
    att_x = np.asarray(inputs["att_x"], np.float32)
    W_att1 = np.asarray(inputs["W_att1"], np.float32)
    b_att1 = np.asarray(inputs["b_att1"], np.float32)
    W_att2 = np.asarray(inputs["W_att2"], np.float32)
    b_att2 = np.asarray(inputs["b_att2"], np.float32)
    W_att = np.asarray(inputs["W_att"], np.float32)
    b_att = np.asarray(inputs["b_att"], np.float32)
    W_weight = np.asarray(inputs["W_weight"], np.float32)
    W_bias = np.asarray(inputs["W_bias"], np.float32)

    w1t = W_att1.T.reshape(4, 128, 256).transpose(1, 0, 2)  # [128, 4, 256]
    w2t = (W_att2 / float(H * W)).T.reshape(2, 128, 256).transpose(1, 0, 2)
    wat = W_att.T.reshape(2, 128, NW).transpose(1, 0, 2)
    wb = W_bias.reshape(2, 128, NW).transpose(1, 0, 2)
    b12 = np.ascontiguousarray((b_att1 + b_att2).reshape(1, 256))
    batt = np.ascontiguousarray(b_att.reshape(1, NW))
    # W_weight rows are [cout, cin, kh, kw] flattened; target [w, ci, (co2 t cout)]
    w5 = W_weight.reshape(COUT, 2, 128, 9, NW)
    wrep = np.ascontiguousarray(
        w5.transpose(4, 2, 1, 3, 0).reshape(NW, 128, KFREE)
    ).astype(bf16)

    # host-prepadded bf16 image: [B, 128 cin-part, 2 cin-chunk, XCOLS]
    xp = np.zeros((B, 128, 2, XCOLS), bf16)
    xv = xp[:, :, :, MARG : MARG + PCOLS].reshape(B, 128, 2, H, WP)[..., :W]
    xv[...] = x.reshape(B, 2, 128, H, W).transpose(0, 2, 1, 3, 4).astype(bf16)
    xp = xp.reshape(B, 128, 2 * XCOLS)

    in_maps = []
    for i in range(NCORES):
        axs = att_x[i * BS : (i + 1) * BS]  # [4, 512]
        attxT = axs.T.reshape(4, 128, BS).transpose(1, 0, 2)  # [128, 4, BS]
        cblob = np.concatenate(
            [
                attxT.reshape(128, 4 * BS)[:, : 4 * BS],
                w1t.reshape(128, 1024),
                w2t.reshape(128, 512),
                wat.reshape(128, 2 * NW),
                wb.reshape(128, 2 * NW),
            ],
            axis=1,
        ).astype(np.float32)
        in_maps.append(
            {
                "cblob": np.ascontiguousarray(cblob),
                "b12": b12,
                "batt": batt,
                "xpad": xp[i * BS : (i + 1) * BS],
                "wrep": wrep,
            }
        )
    return in_maps


def _unpack_y(y_bf16):
    """[B, COUT, NT, NFREE] bf16 padded rows -> [B, COUT, H, W] fp32."""
    y = np.asarray(y_bf16).reshape(B, COUT, NT, RT, WP)[..., 0:W]
    return np.ascontiguousarray(y.reshape(B, COUT, H, W).astype(np.float32))


def kernel(**inputs):
    from concourse.bass_utils import run_bass_kernel_spmd

    if "nc" not in _nc_cache:
        _nc_cache["nc"] = _build_nc()
    in_maps = _prep_inputs(inputs)
    res = run_bass_kernel_spmd(_nc_cache["nc"], in_maps, core_ids=list(range(NCORES)))
    y = np.concatenate([np.asarray(res.results[i]["y"]) for i in range(NCORES)], axis=0)
    return _unpack_y(y)


# revision 19
# speedup vs baseline: 1.3466x; 1.0080x over previous
"""CondConv2d Trainium2 kernel (bf16 data path).

Data-parallel over batch: 32 samples -> 8 cores x 4 samples.
Per core:
  - x arrives host-prepadded in bf16 ([128, 2, XCOLS] per sample, zero
    margins, 57-wide rows whose single pad column doubles as the next
    row's left pad), one contiguous DMA per sample; the expert table
    arrives bf16 and stays SBUF-resident (8 x [128, 4608]).
  - per-sample attention: pooled sums reduce on the DVE over the padded
    image (pad zeros don't change the sum), tiny per-sample MLP on the
    PE, softmax on DVE/ACT, attention row broadcast to all partitions
    via a rank-1 PE matmul.
  - per-sample kernels K_s = sum_w att[s,w] * W_w on the DVE as a
    tensor_scalar_mul (4x packed-bf16 mode) + tensor_tensor add (2x)
    chain — ~2.6x faster than fused scalar_tensor_tensor, and walrus
    rejects TensorScalarPtr on Pool.
  - 3x3 conv as 18-matmul PSUM accumulation groups (2 cin chunks x 9
    taps) in bf16 (1 cycle/row vs 4 for fp32), tap-outer over 4+3
    interleaved PSUM banks so one stationary load serves several row
    tiles (amortizes LDWEIGHTS, ~30% faster), moving operand = shifted
    windows of the padded image, bias added on ScalarE during the
    PSUM->SBUF drain, bf16 output DMA'd (issued from the ACT engine's
    DGE to keep SP free) as full padded rows; host strips pad cols and
    upcasts to fp32.
  - the whole thing is software-pipelined: each (rep, sample) unit's
    attention/kgen chain is issued, in three stages, at conv-chunk
    boundaries 4/3/2 chunks ahead of its own conv, so the in-order PE
    runs the conv matmul stream without stalls (also across reps in the
    timing build).
"""

import numpy as np

B, CIN, H, W = 32, 256, 56, 56
COUT, KH, KW = 256, 3, 3
ATT_IN, NW = 512, 8
NCORES = 8
BS = B // NCORES  # 4

WP = W + 1                 # 57: padded row width (56 + shared pad col; row
                           # h's pad doubles as row h+1's left pad)
PCOLS = H * WP             # 3192
MARG = WP + 1              # 58: left/right margin so tap windows stay in-bounds
XCOLS = MARG + PCOLS + MARG  # 3308
RT = 8                     # output rows per PSUM tile
NT = H // RT               # 7
NFREE = RT * WP            # 456 (<= 512 fp32 PSUM bank)
KFREE = 2 * 9 * COUT       # 4608 free cols of a generated kernel

_nc_cache = {}


def _split_waits(nc, max_inline=1):
    """Hoist inline sem waits beyond max_inline into standalone NoOps.

    This walrus build rejects instructions carrying more than one inline
    sync wait ("Too many sync wait commands"); standalone single-wait
    NoOps on the same engine are semantically identical and compile.
    """
    import copy

    import concourse.mybir as mybir

    cnt = 0
    for f in nc.m.functions:
        new_blocks = []
        for blk in f.blocks:
            out = []
            for inst in blk.instructions:
                si = inst.sync_info
                waits = list(si.on_wait) if si is not None else []
                if len(waits) > max_inline:
                    keep = waits[-max_inline:]
                    hoist = waits[:-max_inline]
                    for w in hoist:
                        nop = mybir.InstNoOp(name=f"WSPL-{cnt}", ins=[], outs=[])
                        cnt += 1
                        nop.engine = inst.engine
                        nop.sync_info = mybir.SyncInfo(on_wait=[w], on_update=[])
                        out.append(nop)
                    inst.sync_info = mybir.SyncInfo(
                        on_wait=keep, on_update=list(si.on_update)
                    )
                out.append(inst)
            new_blocks.append(copy.replace(blk, instructions=out))
        try:
            f.blocks[:] = new_blocks
        except TypeError:
            for i, nb in enumerate(new_blocks):
                f.blocks[i] = nb
    return cnt


def _build_nc(reps=1, split_waits=True):
    from contextlib import ExitStack

    import concourse.bass as bass
    import concourse.mybir as mybir
    import concourse.tile as tile

    fp32 = mybir.dt.float32
    bf16 = mybir.dt.bfloat16
    AF = mybir.ActivationFunctionType
    ALU = mybir.AluOpType
    AX = mybir.AxisListType

    nc = bass.Bass()

    # cblob packs the [128, ...] fp32 constants: attxT(16) w1t(1024)
    # w2t(512) wat(16) wb(16) -> 1584 cols
    cblob_d = nc.declare_dram_parameter("cblob", [128, 1584], fp32, isOutput=False)
    b12_d = nc.declare_dram_parameter("b12", [1, 256], fp32, isOutput=False)
    batt_d = nc.declare_dram_parameter("batt", [1, NW], fp32, isOutput=False)
    xpad_d = nc.declare_dram_parameter("xpad", [BS, 128, 2 * XCOLS], bf16, isOutput=False)
    wrep_d = nc.declare_dram_parameter("wrep", [NW, 128, KFREE], bf16, isOutput=False)
    y_d = nc.declare_dram_parameter("y", [BS, COUT, NT, NFREE], bf16, isOutput=True)

    with ExitStack() as ctx:
        tc = ctx.enter_context(tile.TileContext(nc))
        constp = ctx.enter_context(tc.tile_pool(name="const", bufs=1))
        wtabp = ctx.enter_context(tc.tile_pool(name="wtab", bufs=1))
        kgenp = ctx.enter_context(tc.tile_pool(name="kgen", bufs=1))
        xpadp = ctx.enter_context(tc.tile_pool(name="xpad", bufs=1))
        tmpp = ctx.enter_context(tc.tile_pool(name="tmp", bufs=2))
        osbp = ctx.enter_context(tc.tile_pool(name="osb", bufs=4))
        psum_s = ctx.enter_context(tc.tile_pool(name="psum_s", bufs=1, space="PSUM"))
        psum_c = ctx.enter_context(tc.tile_pool(name="psum_c", bufs=7, space="PSUM"))

        # --- constants (loaded once, reused across reps)
        # consts ride the Activation engine's DGE so SP's queue starts on
        # xpad/wtab (the critical path) immediately
        cblob = constp.tile([128, 1584], fp32)
        nc.scalar.dma_start(cblob[:], cblob_d[:])
        b12 = constp.tile([1, 256], fp32)
        nc.scalar.dma_start(b12[:], b12_d[:])
        batt = constp.tile([1, NW], fp32)
        nc.scalar.dma_start(batt[:], batt_d[:])
        attxT = cblob[:, 0:16].rearrange("p (k s) -> p k s", k=4)
        w1t = cblob[:, 16:1040].rearrange("p (k m) -> p k m", k=4)
        w2t = cblob[:, 1040:1552].rearrange("p (k m) -> p k m", k=2)
        wat = cblob[:, 1552:1568].rearrange("p (k m) -> p k m", k=2)
        wb = cblob[:, 1568:1584].rearrange("p (k m) -> p k m", k=2)
        ones11 = constp.tile([1, 1], fp32)
        nc.vector.memset(ones11[:], 1.0)
        ones_bc = constp.tile([1, 128], fp32)
        nc.vector.memset(ones_bc[:], 1.0)

        # --- persistent state
        kgen = [kgenp.tile([128, KFREE], bf16, tag=f"kgen{s}", name=f"kgen{s}") for s in range(BS)]
        xpad = [
            xpadp.tile([128, 2, XCOLS], bf16, tag=f"xpad{s}", name=f"xpad{s}")
            for s in range(BS)
        ]
        wtab = [
            wtabp.tile([128, KFREE], bf16, tag=f"wtab{w}", name=f"wtab{w}")
            for w in range(NW)
        ]
        pooledT = constp.tile([128, 2, BS], fp32)
        mixT = constp.tile([128, 2, BS], fp32)
        attbc = constp.tile([128, BS * NW], fp32)
        biasall = constp.tile([128, 2, BS], fp32)
        p3s = constp.tile([1, NW], fp32)
        mx = constp.tile([1, 1], fp32)
        nmx = constp.tile([1, 1], fp32)
        ex = constp.tile([1, NW], fp32)
        sm = constp.tile([1, 1], fp32)
        rcp = constp.tile([1, 1], fp32)
        att1 = constp.tile([1, NW], fp32)

        def att_ap(s, w):
            return attbc[:, s * NW + w : s * NW + w + 1]

        def issue_dmas(r):
            # SP issue order = critical-path order: xpad[0], xpad[1] (first
            # two attention chains), the full expert table, then the rest
            for s in (0, 1):
                nc.sync.dma_start(
                    xpad[s][:], xpad_d[s].rearrange("p (c q) -> p c q", c=2)
                )
            for w in range(NW):
                nc.sync.dma_start(wtab[w][:], wrep_d[w])
            for s in (2, 3):
                nc.sync.dma_start(
                    xpad[s][:], xpad_d[s].rearrange("p (c q) -> p c q", c=2)
                )

        if True:
            def chain_stage1(r, s):
                """DMA issue, pooled sums (both halves on ACT via accum_out,
                keeping the DVE queue clear for kgen FMAs), MLP layer 1."""
                if s == 0:
                    issue_dmas(r)
                # pooled sums on DVE: keeps the ACT engine's instruction
                # stream (PSUM drains) free of Copy-table switches
                for co in range(2):
                    nc.vector.reduce_sum(
                        pooledT[:, co : co + 1, s : s + 1],
                        xpad[s][:, co : co + 1, :], axis=AX.X,
                    )
                for c2 in range(2):
                    pm = psum_s.tile([128, 1], fp32, tag="mlps", name="pm")
                    nc.tensor.matmul(
                        pm[:], w1t[:, 0, c2 * 128 : (c2 + 1) * 128],
                        attxT[:, 0, s : s + 1], start=True, stop=False,
                    )
                    for ko in range(1, 4):
                        nc.tensor.matmul(
                            pm[:], w1t[:, ko, c2 * 128 : (c2 + 1) * 128],
                            attxT[:, ko, s : s + 1], start=False, stop=False,
                        )
                    for jo in range(2):
                        nc.tensor.matmul(
                            pm[:], w2t[:, jo, c2 * 128 : (c2 + 1) * 128],
                            pooledT[:, jo, s : s + 1], start=False, stop=False,
                        )
                    nc.tensor.matmul(
                        pm[:], b12[0:1, c2 * 128 : (c2 + 1) * 128], ones11[0:1, :],
                        start=False, stop=True,
                    )
                    nc.scalar.activation(mixT[:, c2, s : s + 1], pm[:], AF.Tanh)

            def chain_stage2(r, s):
                """Worker logits + softmax (tiny ops, partition 0)."""
                p3 = psum_s.tile([1, NW], fp32, tag="mlps", name="p3")
                nc.tensor.matmul(
                    p3[:], mixT[:, 0, s : s + 1], wat[:, 0, :], start=True, stop=False
                )
                nc.tensor.matmul(
                    p3[:], mixT[:, 1, s : s + 1], wat[:, 1, :], start=False, stop=False
                )
                nc.tensor.matmul(
                    p3[:], ones11[0:1, :], batt[0:1, :], start=False, stop=True
                )
                nc.vector.tensor_copy(p3s[:], p3[:])
                nc.vector.reduce_max(mx[:], p3s[:], axis=AX.X)
                nc.vector.tensor_scalar_mul(nmx[:], mx[:], -1.0)
                nc.scalar.activation(ex[:], p3s[:], AF.Exp, bias=nmx[:], scale=1.0)
                nc.vector.reduce_sum(sm[:], ex[:], axis=AX.X)
                nc.vector.reciprocal(rcp[:], sm[:])
                nc.vector.tensor_scalar_mul(att1[:], ex[:], rcp[:])

            def chain_stage3(r, s):
                """Broadcast attention, then kgen[s] + biasall[s] on DVE.

                tensor_scalar_mul runs in 4x mode and tensor_tensor add in
                2x mode for packed bf16 — together ~2.6x faster than the
                fused scalar_tensor_tensor chain (and walrus rejects
                TensorScalarPtr on Pool)."""
                pbc = psum_s.tile([128, NW], fp32, tag="mlps", name="pbc")
                nc.tensor.matmul(pbc[:], ones_bc[0:1, :], att1[0:1, :], start=True, stop=True)
                nc.vector.tensor_copy(attbc[:, s * NW : (s + 1) * NW], pbc[:])

                nc.vector.tensor_scalar_mul(
                    kgen[s][:], wtab[0][:], att_ap(s, 0)
                )
                for w in range(1, NW):
                    tmp = tmpp.tile([128, KFREE], bf16)
                    nc.vector.tensor_scalar_mul(
                        tmp[:], wtab[w][:], att_ap(s, w)
                    )
                    nc.vector.tensor_tensor(
                        kgen[s][:], kgen[s][:], tmp[:], ALU.add
                    )

                nc.vector.tensor_scalar_mul(
                    biasall[:, :, s : s + 1], wb[:, :, 0:1], att_ap(s, 0)
                )
                for w in range(1, NW):
                    nc.vector.scalar_tensor_tensor(
                        biasall[:, :, s : s + 1], wb[:, :, w : w + 1], att_ap(s, w),
                        biasall[:, :, s : s + 1], ALU.mult, ALU.add,
                    )

            def conv_chunk(s, c):
                """One cout-half of sample s: tap-outer over interleaved PSUM
                groups so one stationary weight load serves several row-tile
                matmuls (amortizes LDWEIGHTS ~7x vs tap-inner order)."""
                kgv = kgen[s][:].rearrange("p (co t m) -> p co t m", co=2, t=9)
                xb = xpad[s]
                bias_ap = biasall[:, c, s : s + 1]
                for rts in ((0, 1, 2, 3), (4, 5, 6)):
                    ps = {rt: psum_c.tile([128, NFREE], fp32, tag="psc", name="psc") for rt in rts}
                    for co in range(2):
                        for t in range(9):
                            dh, dw = t // 3 - 1, t % 3 - 1
                            for rt in rts:
                                qs = MARG + rt * RT * WP + dh * WP + dw
                                nc.tensor.matmul(
                                    ps[rt][:],
                                    kgv[:, co, t, c * 128 : (c + 1) * 128],
                                    xb[:, co, qs : qs + NFREE],
                                    start=(co == 0 and t == 0),
                                    stop=(co == 1 and t == 8),
                                    skip_group_check=True,
                                )
                    for rt in rts:
                        ob = osbp.tile([128, NFREE], bf16)
                        nc.scalar.activation(
                            ob[:], ps[rt][:], AF.Identity, bias=bias_ap, scale=1.0
                        )
                        nc.scalar.dma_start(
                            y_d[s, c * 128 : (c + 1) * 128, rt, :], ob[:]
                        )

            # software-pipelined schedule over all (rep, sample) units: each
            # unit's attention/kgen chain is spread over three conv-chunk
            # boundaries ahead of its own conv — across rep boundaries too —
            # so the in-order PE never stalls on a late attention chain,
            # cross-engine softmax round-trip, or table reload.
            seq = [(r, s) for r in range(reps) for s in range(BS)]
            N = len(seq)
            stages = {1: chain_stage1, 2: chain_stage2, 3: chain_stage3}
            events = [[] for _ in range(2 * N)]
            for k in range(N):
                for st, off in ((1, 2 * k - 4), (2, 2 * k - 3), (3, 2 * k - 2)):
                    events[max(0, off)].append((k, st))
            for b in range(2 * N):
                for k, st in events[b]:
                    stages[st](*seq[k])
                conv_chunk(seq[b // 2][1], b % 2)

    if split_waits:
        _split_waits(nc)
    return nc


def _prep_inputs(inputs):
    import ml_dtypes

    bf16 = ml_dtypes.bfloat16

    x = np.asarray(inputs["x"], np.float32)
    att_x = np.asarray(inputs["att_x"], np.float32)
    W_att1 = np.asarray(inputs["W_att1"], np.float32)
    b_att1 = np.asarray(inputs["b_att1"], np.float32)
    W_att2 = np.asarray(inputs["W_att2"], np.float32)
    b_att2 = np.asarray(inputs["b_att2"], np.float32)
    W_att = np.asarray(inputs["W_att"], np.float32)
    b_att = np.asarray(inputs["b_att"], np.float32)
    W_weight = np.asarray(inputs["W_weight"], np.float32)
    W_bias = np.asarray(inputs["W_bias"], np.float32)

    w1t = W_att1.T.reshape(4, 128, 256).transpose(1, 0, 2)  # [128, 4, 256]
    w2t = (W_att2 / float(H * W)).T.reshape(2, 128, 256).transpose(1, 0, 2)
    wat = W_att.T.reshape(2, 128, NW).transpose(1, 0, 2)
    wb = W_bias.reshape(2, 128, NW).transpose(1, 0, 2)
    b12 = np.ascontiguousarray((b_att1 + b_att2).reshape(1, 256))
    batt = np.ascontiguousarray(b_att.reshape(1, NW))
    # W_weight rows are [cout, cin, kh, kw] flattened; target [w, ci, (co2 t cout)]
    w5 = W_weight.reshape(COUT, 2, 128, 9, NW)
    wrep = np.ascontiguousarray(
        w5.transpose(4, 2, 1, 3, 0).reshape(NW, 128, KFREE)
    ).astype(bf16)

    # host-prepadded bf16 image: [B, 128 cin-part, 2 cin-chunk, XCOLS]
    xp = np.zeros((B, 128, 2, XCOLS), bf16)
    xv = xp[:, :, :, MARG : MARG + PCOLS].reshape(B, 128, 2, H, WP)[..., :W]
    xv[...] = x.reshape(B, 2, 128, H, W).transpose(0, 2, 1, 3, 4).astype(bf16)
    xp = xp.reshape(B, 128, 2 * XCOLS)

    in_maps = []
    for i in range(NCORES):
        axs = att_x[i * BS : (i + 1) * BS]  # [4, 512]
        attxT = axs.T.reshape(4, 128, BS).transpose(1, 0, 2)  # [128, 4, BS]
        cblob = np.concatenate(
            [
                attxT.reshape(128, 4 * BS)[:, : 4 * BS],
                w1t.reshape(128, 1024),
                w2t.reshape(128, 512),
                wat.reshape(128, 2 * NW),
                wb.reshape(128, 2 * NW),
            ],
            axis=1,
        ).astype(np.float32)
        in_maps.append(
            {
                "cblob": np.ascontiguousarray(cblob),
                "b12": b12,
                "batt": batt,
                "xpad": xp[i * BS : (i + 1) * BS],
                "wrep": wrep,
            }
        )
    return in_maps


def _unpack_y(y_bf16):
    """[B, COUT, NT, NFREE] bf16 padded rows -> [B, COUT, H, W] fp32."""
    y = np.asarray(y_bf16).reshape(B, COUT, NT, RT, WP)[..., 0:W]
    return np.ascontiguousarray(y.reshape(B, COUT, H, W).astype(np.float32))


def kernel(**inputs):
    from concourse.bass_utils import run_bass_kernel_spmd

    if "nc" not in _nc_cache:
        _nc_cache["nc"] = _build_nc()
    in_maps = _prep_inputs(inputs)
    res = run_bass_kernel_spmd(_nc_cache["nc"], in_maps, core_ids=list(range(NCORES)))
    y = np.concatenate([np.asarray(res.results[i]["y"]) for i in range(NCORES)], axis=0)
    return _unpack_y(y)
